# revision 20
# baseline (speedup 1.0000x reference)
"""Trainium2 Bass kernel for nn_MultiHeadAttention (RoPE MHA, B=2 S=2048 E=1024 H=16).

Sharding: tensor-parallel over heads — 2 heads per core on 8 cores. Each core
computes its heads' q/k/v projections, RoPE, attention, and the partial output
projection (its rows of Wo); the host sums the 8 partials and adds bo.

Device layouts: q/k as [d, token] (transposed) so attention scores come out as
[ks, qs]; softmax's row-sum falls out of the same matmul that computes ctx via
a shared ones column in v ([v0 | 1 1 | v1]: h0 reads cols 0:65 -> Z at row 64,
h1 reads cols 65:130 -> Z at row 0). rotate_half is a signed-permutation
matmul. v is projected transposed (full-width matmuls) and PE-transposed.

Schedule: chunk-gated waves. Attention kt tiles start as soon as the proj
chunk covering their keys is done, so the exp stream on ACT starts ~9us in.
The kt loop is software-pipelined (ctx lags scores by one kt so PE never
waits on ACT), and proj/out-proj work is interleaved between kt units to
fill PE slack. PSUM: pss double-buffer (2x2 banks, dedicated), psc (2x1,
block-persistent), everything else rotates through 2x1 shared slots.
"""

import os
import sys
from contextlib import ExitStack

import numpy as np

for _p in ("/opt/trn_rl_repo", "/opt/pypackages"):
    if _p not in sys.path and os.path.isdir(_p):
        sys.path.append(_p)

import concourse.bass as bass
import concourse.mybir as mybir
import concourse.tile as tile
from concourse import bacc
from concourse import bass_utils
from concourse.masks import make_identity

F32 = mybir.dt.float32
AF = mybir.ActivationFunctionType
OP = mybir.AluOpType

B = 2
S = 2048
E = 1024
H = 16
D = 64
N_CORES = 8
HPC = H // N_CORES  # heads per core = 2
HD = HPC * D  # 128

MM_MODE = os.environ.get("MHA_MM_MODE", "bf16")

LAST_RESULTS = None  # BassKernelResults of the most recent run (for test harness)
_NC_CACHE = {}

VW = 132  # v tile row width: v0(64) | ones | pad | v1(64) | ones | pad


def build_mha_nc(mm_mode):
    T = B * S
    TC = 512  # token chunk for projections
    NCH = T // TC  # 8
    QC = 512  # query chunk in attention
    NQC = S // QC  # 4
    NKT = S // 128  # 16 key tiles per batch
    KE = E // 128  # 8 contraction tiles for projections

    dt_in = {"bf16": mybir.dt.bfloat16, "f32r": mybir.dt.float32r, "f32": F32}[mm_mode]

    nc = bacc.Bacc(None, target_bir_lowering=False, debug=False)

    xT = nc.dram_tensor("xT", [E, T], dt_in, kind="ExternalInput")
    wq = nc.dram_tensor("wq", [E, HD], dt_in, kind="ExternalInput")
    wk = nc.dram_tensor("wk", [E, HD], dt_in, kind="ExternalInput")
    wv = nc.dram_tensor("wv", [E, HD], dt_in, kind="ExternalInput")
    bq = nc.dram_tensor("bq", [HD, 1], F32, kind="ExternalInput")
    bk = nc.dram_tensor("bk", [HD, 1], F32, kind="ExternalInput")
    bv = nc.dram_tensor("bv", [HD, 1], F32, kind="ExternalInput")
    wo = nc.dram_tensor("wo", [HD, E], dt_in, kind="ExternalInput")
    cosT = nc.dram_tensor("cosT", [D, S], F32, kind="ExternalInput")
    sinT = nc.dram_tensor("sinT", [D, S], F32, kind="ExternalInput")
    rot = nc.dram_tensor("rot", [HD, HD], dt_in, kind="ExternalInput")
    yp = nc.dram_tensor("yp", [T, E], dt_in, kind="ExternalOutput")
    debug = os.environ.get("MHA_DEBUG", "") == "1"
    if debug:
        dbg_q = nc.dram_tensor("dbg_q", [HD, T], dt_in, kind="ExternalOutput")
        dbg_k = nc.dram_tensor("dbg_k", [HD, T], dt_in, kind="ExternalOutput")
        dbg_v = nc.dram_tensor("dbg_v", [T // 128, 128, VW], dt_in, kind="ExternalOutput")
        dbg_cp = nc.dram_tensor("dbg_cp", [B, HD, S], dt_in, kind="ExternalOutput")

    scale = 1.0 / np.sqrt(D)

    with tile.TileContext(nc) as tc, ExitStack() as ctx:
        const = ctx.enter_context(tc.tile_pool(name="const", bufs=1))
        xt_pool = ctx.enter_context(tc.tile_pool(name="xt", bufs=2 * KE))
        cs_pool = ctx.enter_context(tc.tile_pool(name="cs", bufs=4))
        qkraw_pool = ctx.enter_context(tc.tile_pool(name="qkraw", bufs=4))
        rope_tmp = ctx.enter_context(tc.tile_pool(name="ropetmp", bufs=4))
        persist = ctx.enter_context(tc.tile_pool(name="persist", bufs=1))
        exps_pool = ctx.enter_context(tc.tile_pool(name="exps", bufs=8))
        zr_pool = ctx.enter_context(tc.tile_pool(name="zr", bufs=6))
        zb_pool = ctx.enter_context(tc.tile_pool(name="zb", bufs=6))
        osb_pool = ctx.enter_context(tc.tile_pool(name="osb", bufs=6))
        csh_pool = ctx.enter_context(tc.tile_pool(name="csh", bufs=6))
        dram = ctx.enter_context(tc.tile_pool(name="dram", bufs=8, space="DRAM"))

        # PSUM: pss 2x2 banks (dedicated), psc 2x1 (block-persistent),
        # shared 2x1 rotation for psq/psk/psv/psrot/pvt/pso.
        ps_s = ctx.enter_context(tc.tile_pool(name="ps_s", bufs=2, space="PSUM"))
        ps_c = ctx.enter_context(tc.tile_pool(name="ps_c", bufs=2, space="PSUM"))
        ps_w = ctx.enter_context(tc.tile_pool(name="ps_w", bufs=2, space="PSUM"))

        # ---- constants to SBUF (gpsimd queue; off the sync DMA path) ----
        def load_const(name, dram_t, shape, dt):
            t = const.tile(shape, dt, name=name, tag=name)
            nc.gpsimd.dma_start(t[:], dram_t.ap())
            return t

        wq_sb = [None] * KE
        wk_sb = [None] * KE
        wv_sb = [None] * KE
        for k in range(KE):
            for nm, dr, arr in (("wq", wq, wq_sb), ("wk", wk, wk_sb), ("wv", wv, wv_sb)):
                t = const.tile([128, HD], dt_in, name=f"{nm}_{k}", tag=f"{nm}_{k}")
                nc.gpsimd.dma_start(t[:], dr.ap()[128 * k : 128 * (k + 1), :])
                arr[k] = t
        bq_sb = load_const("bq_sb", bq, [HD, 1], F32)
        bk_sb = load_const("bk_sb", bk, [HD, 1], F32)
        bv_sb = load_const("bv_sb", bv, [HD, 1], F32)
        wo_sb = load_const("wo_sb", wo, [HD, E], dt_in)
        rot_sb = load_const("rot_sb", rot, [HD, HD], dt_in)
        ident = const.tile([128, 128], dt_in, name="ident", tag="ident")
        make_identity(nc, ident)

        # ---- persistent intermediates ----
        q_rope = persist.tile([HD, T], dt_in, name="q_rope", tag="q_rope")
        k_rope = persist.tile([HD, T], dt_in, name="k_rope", tag="k_rope")
        # v tiles: [tokens(128), v0(64) | ones(2) | v1(64)] per 128-token tile
        v_sb = []
        for i in range(T // 128):
            t = persist.tile([128, VW], dt_in, name=f"v_{i}", tag=f"v_{i}")
            nc.vector.memset(t[:, 64:65], 1.0)
            nc.vector.memset(t[:, 130:131], 1.0)
            v_sb.append(t)
        ctx_pack = {}
        for b in range(B):
            ctx_pack[b] = persist.tile([HD, S], dt_in, name=f"ctxp_{b}", tag=f"ctxp_{b}")

        # ---- stage 1: projection chunk, split into 3 parts (q, k, v) ----
        def part_qk(c, nm):
            c0 = TC * c
            if nm == "q":  # loads for the whole chunk ride with the q part
                for k in range(KE):
                    t = xt_pool.tile([128, TC], dt_in, name=f"xt_{c}_{k}", tag="xt")
                    nc.sync.dma_start(t[:], xT.ap()[128 * k : 128 * (k + 1), c0 : c0 + TC])
                    xt_cur[c] = xt_cur.get(c, [])
                    xt_cur[c].append(t)
                s0 = c0 % S
                for key, dr in (("cos", cosT), ("sin", sinT)):
                    t = cs_pool.tile([HD, TC], F32, name=f"{key}_c", tag=f"{key}_c")
                    dr_ap = dr.ap()
                    nc.sync.dma_start(
                        t[:],
                        bass.AP(
                            tensor=dr_ap.tensor,
                            offset=dr_ap.offset + s0,
                            ap=[[0, HPC], [S, D], [1, TC]],
                        ),
                    )
                    cs_cur[(c, key)] = t
            xt = xt_cur[c]
            w_sb, b_sb, out = {
                "q": (wq_sb, bq_sb, q_rope),
                "k": (wk_sb, bk_sb, k_rope),
            }[nm]
            ps = ps_w.tile([HD, TC], F32, name=f"ps{nm}", tag="ps_w")
            for k in range(KE):
                nc.tensor.matmul(
                    ps[:], w_sb[k][:], xt[k][:], start=(k == 0), stop=(k == KE - 1)
                )
            raw = qkraw_pool.tile([HD, TC], dt_in, name=f"{nm}raw", tag="qkraw")
            nc.vector.tensor_scalar_add(raw[:], ps[:], b_sb[:, 0:1])
            psrot = ps_w.tile([HD, TC], F32, name="psrot", tag="ps_w")
            nc.tensor.matmul(psrot[:], rot_sb[:], raw[:], start=True, stop=True)
            cprod = rope_tmp.tile([HD, TC], F32, name="cprod", tag="ropetmp")
            nc.vector.tensor_tensor(cprod[:], raw[:], cs_cur[(c, "cos")][:], op=OP.mult)
            sprod = rope_tmp.tile([HD, TC], F32, name="sprod", tag="ropetmp")
            nc.vector.tensor_tensor(sprod[:], psrot[:], cs_cur[(c, "sin")][:], op=OP.mult)
            nc.vector.tensor_tensor(out[:, c0 : c0 + TC], cprod[:], sprod[:], op=OP.add)

        def part_v(c):
            c0 = TC * c
            xt = xt_cur[c]
            psv = ps_w.tile([HD, TC], F32, name="psv", tag="ps_w")
            for k in range(KE):
                nc.tensor.matmul(
                    psv[:], wv_sb[k][:], xt[k][:], start=(k == 0), stop=(k == KE - 1)
                )
            vraw = qkraw_pool.tile([HD, TC], dt_in, name="vraw", tag="qkraw")
            nc.vector.tensor_scalar_add(vraw[:], psv[:], bv_sb[:, 0:1])
            for j in range(TC // 128):
                pvt = ps_w.tile([128, 128], dt_in, name="pvt", tag="ps_w")
                nc.tensor.transpose(pvt[:], vraw[:, 128 * j : 128 * (j + 1)], ident[:])
                vt = v_sb[(c0 + 128 * j) // 128]
                vt_ap = vt[:]
                # one strided copy: psum cols {0:64,64:128} -> vt cols {0:64,66:130}
                dst = bass.AP(
                    tensor=vt_ap.tensor,
                    offset=vt_ap.offset,
                    ap=[vt_ap.ap[0], [66, 2], [1, 64]],
                )
                pvt_ap = pvt[:]
                src = bass.AP(
                    tensor=pvt_ap.tensor,
                    offset=pvt_ap.offset,
                    ap=[pvt_ap.ap[0], [64, 2], [1, 64]],
                )
                nc.vector.tensor_copy(dst, src)

        xt_cur = {}
        cs_cur = {}

        # ---- stage 2: attention, software-pipelined kt loop ----
        def open_block():
            # h0: ctx rows 0:64, Z row 64 ; h1: Z row 0, ctx rows 1:65
            return [
                ps_c.tile([D + 1, QC], F32, name=f"psctx{h}", tag="ps_c")
                for h in range(HPC)
            ]

        def kt_scores(b, qc, kt):
            t0 = b * S
            q0 = t0 + QC * qc
            k0 = t0 + 128 * kt
            pss = ps_s.tile([128, HPC * QC], F32, name="pss", tag="ps_s")
            for h in range(HPC):
                nc.tensor.matmul(
                    pss[:, QC * h : QC * (h + 1)],
                    k_rope[D * h : D * (h + 1), k0 : k0 + 128],
                    q_rope[D * h : D * (h + 1), q0 : q0 + QC],
                    start=True, stop=True,
                )
            ex = exps_pool.tile([128, HPC * QC], dt_in, name="ex", tag="exps")
            nc.scalar.activation(ex[:], pss[:], AF.Exp, scale=scale)
            return ex

        def kt_ctx(b, qc, kt, psc, ex):
            t0 = b * S
            k0 = t0 + 128 * kt
            vt = v_sb[k0 // 128]
            for h in range(HPC):
                nc.tensor.matmul(
                    psc[h][:],
                    vt[:, 66 * h : 66 * h + 65],
                    ex[:, QC * h : QC * (h + 1)],
                    start=(kt == 0), stop=(kt == NKT - 1),
                )

        def finish_block(b, qc, psc):
            # evict unnormalized ctx+Z (f32), 1/Z via fast approx, DRAM-bounce
            # broadcast, normalize-multiply into ctx_pack (bf16)
            for h in range(HPC):
                cun = csh_pool.tile([D + 1, QC], F32, name="cun", tag="csh")
                nc.vector.tensor_copy(cun[:], psc[h][:])
                zrow = zr_pool.tile([1, QC], F32, name="zrow", tag="zrow")
                nc.vector.tensor_copy(zrow[:], cun[D : D + 1, :])
                zr = zr_pool.tile([1, QC], F32, name="zrec", tag="zr")
                nc.vector.reciprocal_approx_fast(zr[:], zrow[:])
                zd = dram.tile([QC], F32, name="zd", tag="zd")
                nc.gpsimd.dma_start(zd[:], zr[:])
                zb = zb_pool.tile([D, QC], F32, name="zb", tag="zb")
                zd_ap = zd[:]
                nc.gpsimd.dma_start(
                    zb[:],
                    bass.AP(
                        tensor=zd_ap.tensor, offset=zd_ap.offset, ap=[[0, D], [1, QC]]
                    ),
                )
                if h == 0:
                    nc.vector.tensor_tensor(
                        ctx_pack[b][0:D, QC * qc : QC * (qc + 1)],
                        cun[0:D, :], zb[:], op=OP.mult,
                    )
                else:
                    csh = csh_pool.tile([D, QC], dt_in, name="csh2", tag="csh2")
                    nc.vector.tensor_tensor(csh[:], cun[0:D, :], zb[:], op=OP.mult)
                    nc.scalar.dma_start(
                        ctx_pack[b][D : 2 * D, QC * qc : QC * (qc + 1)], csh[:]
                    )

        # ---- stage 3: one out-projection tile (j, e) of batch b ----
        def out_tile(b, j, e):
            t0 = b * S
            pso = ps_w.tile([128, 512], F32, name="pso", tag="ps_w")
            nc.tensor.matmul(
                pso[:],
                ctx_pack[b][:, 128 * j : 128 * (j + 1)],
                wo_sb[:, 512 * e : 512 * (e + 1)],
                start=True, stop=True,
            )
            osb = osb_pool.tile([128, 512], dt_in, name="osb", tag="osb")
            nc.vector.tensor_copy(osb[:], pso[:])
            eng = nc.sync if e == 0 else nc.gpsimd
            eng.dma_start(
                yp.ap()[t0 + 128 * j : t0 + 128 * (j + 1), 512 * e : 512 * (e + 1)],
                osb[:],
            )

        # ---- emission: chunk-gated waves with interleaved fillers ----
        def emit_wave(kt_units, fillers):
            """Interleave filler callables evenly among kt unit callables."""
            nf, nk = len(fillers), len(kt_units)
            fi = 0
            for i, ku in enumerate(kt_units):
                ku()
                want = (i + 1) * nf // max(nk, 1)
                while fi < want:
                    fillers[fi]()
                    fi += 1
            while fi < nf:
                fillers[fi]()
                fi += 1

        def block_units(b, qc, psc):
            """Software-pipelined kt units: scores(kt) then ctx(kt-1)."""
            state = {}
            units = []

            def mk_scores(kt):
                def f():
                    state[kt] = kt_scores(b, qc, kt)
                return f

            def mk_ctx(kt):
                def f():
                    kt_ctx(b, qc, kt, psc, state.pop(kt))
                return f

            units.append(mk_scores(0))
            for kt in range(1, NKT):
                units.append(mk_scores(kt))
                units.append(mk_ctx(kt - 1))
            units.append(mk_ctx(NKT - 1))
            return units

        def _hp(f):
            def g():
                with tc.high_priority():
                    f()
            return g

        def chunk_parts(c):
            return [
                _hp(lambda c=c: part_qk(c, "q")),
                _hp(lambda c=c: part_qk(c, "k")),
                _hp(lambda c=c: part_v(c)),
            ]

        def outproj_parts(b, qc):
            JT = S // 128
            tiles = range(JT * qc // NQC, JT * (qc + 1) // NQC)
            return [
                (lambda b=b, j=j, e=e: out_tile(b, j, e))
                for j in tiles
                for e in range(E // 512)
            ]

        # chunk 0 alone (nothing to overlap yet)
        for p in chunk_parts(0):
            p()
        # block (0,0): kt range gated on chunks; chunk c+1 interleaves with
        # the kt wave that chunk c enabled
        psc00 = open_block()
        u = block_units(0, 0, psc00)
        # units list: index of scores(kt) = 2*kt-? -> split by kt groups of 4:
        # scores(0..3)+ctx(0..2) ~ units[0:7], then 8 units per 4-kt group
        emit_wave(u[0:7], chunk_parts(1))
        emit_wave(u[7:15], chunk_parts(2))
        emit_wave(u[15:23], chunk_parts(3))
        emit_wave(u[23:32], chunk_parts(4))
        finish_block(0, 0, psc00)

        # blocks (0,1..3): full kt runs, interleaved with chunk 5..7 and
        # lagging out-proj of the previous block
        for qc in range(1, NQC):
            psc = open_block()
            fillers = []
            if qc + 4 < NCH:
                fillers += chunk_parts(qc + 4)
            fillers += outproj_parts(0, qc - 1)
            emit_wave(block_units(0, qc, psc), fillers)
            finish_block(0, qc, psc)

        # batch 1 blocks; out-proj keeps lagging one block
        prev = [(0, NQC - 1)]
        for qc in range(NQC):
            psc = open_block()
            pb, pqc = prev[-1]
            emit_wave(block_units(1, qc, psc), outproj_parts(pb, pqc))
            finish_block(1, qc, psc)
            prev.append((1, qc))
        # final out-proj tail
        for p in outproj_parts(1, NQC - 1):
            p()

        if debug:
            nc.sync.dma_start(dbg_q.ap(), q_rope[:])
            nc.sync.dma_start(dbg_k.ap(), k_rope[:])
            for i in range(T // 128):
                nc.sync.dma_start(dbg_v.ap()[i], v_sb[i][:])
            for b in range(B):
                nc.sync.dma_start(dbg_cp.ap()[b], ctx_pack[b][:])

    nc.compile()
    return nc


def _rope_tables():
    inv_freq = 1.0 / (10000.0 ** (np.arange(0, D, 2, dtype=np.float32) / D))
    t = np.arange(S, dtype=np.float32)
    freqs = np.outer(t, inv_freq).astype(np.float32)
    emb = np.concatenate([freqs, freqs], axis=-1)
    return np.cos(emb).astype(np.float32), np.sin(emb).astype(np.float32)


def _rot_matrix():
    R = np.zeros((HD, HD), np.float32)
    for hh in range(HPC):
        for do in range(D):
            po = D * hh + do
            if do < D // 2:
                R[D * hh + do + D // 2, po] = -1.0
            else:
                R[D * hh + do - D // 2, po] = 1.0
    return R


def kernel(x, Wq, bq, Wk, bk, Wv, bv, Wo, bo):
    global LAST_RESULTS
    import ml_dtypes

    x = np.asarray(x, dtype=np.float32)
    Wq, bq = np.asarray(Wq, np.float32), np.asarray(bq, np.float32)
    Wk, bk = np.asarray(Wk, np.float32), np.asarray(bk, np.float32)
    Wv, bv = np.asarray(Wv, np.float32), np.asarray(bv, np.float32)
    Wo, bo = np.asarray(Wo, np.float32), np.asarray(bo, np.float32)

    mode = MM_MODE
    dt_np = ml_dtypes.bfloat16 if mode == "bf16" else np.float32
    T = B * S

    if mode not in _NC_CACHE:
        _NC_CACHE[mode] = build_mha_nc(mode)
    nc = _NC_CACHE[mode]

    xT = np.ascontiguousarray(x.reshape(T, E).T).astype(dt_np)
    cos, sin = _rope_tables()
    cosT = np.ascontiguousarray(cos.T).astype(np.float32)
    sinT = np.ascontiguousarray(sin.T).astype(np.float32)
    R = _rot_matrix().astype(dt_np)

    in_maps = []
    for c in range(N_CORES):
        sl = slice(HD * c, HD * (c + 1))
        in_maps.append(
            {
                "xT": xT,
                "wq": np.ascontiguousarray(Wq[:, sl]).astype(dt_np),
                "wk": np.ascontiguousarray(Wk[:, sl]).astype(dt_np),
                "wv": np.ascontiguousarray(Wv[:, sl]).astype(dt_np),
                "bq": np.ascontiguousarray(bq[sl][:, None]).astype(np.float32),
                "bk": np.ascontiguousarray(bk[sl][:, None]).astype(np.float32),
                "bv": np.ascontiguousarray(bv[sl][:, None]).astype(np.float32),
                "wo": np.ascontiguousarray(Wo[sl, :]).astype(dt_np),
                "cosT": cosT,
                "sinT": sinT,
                "rot": R,
            }
        )

    res = bass_utils.run_bass_kernel_spmd(nc, in_maps, core_ids=list(range(N_CORES)))
    LAST_RESULTS = res

    out = np.zeros((T, E), np.float64)
    for c in range(N_CORES):
        out += res.results[c]["yp"].astype(np.float64)
    out += bo.astype(np.float64)
    return out.astype(np.float32).reshape(B, S, E)


# revision 21
# speedup vs baseline: 1.0197x; 1.0197x over previous
"""Trainium2 Bass kernel for nn_MultiHeadAttention (RoPE MHA, B=2 S=2048 E=1024 H=16).

Sharding: tensor-parallel over heads — 2 heads per core on 8 cores. Each core
computes its heads' q/k/v projections, RoPE, attention, and the partial output
projection (its rows of Wo); the host sums the 8 partials and adds bo.

Device layouts: q/k as [d, token] (transposed) so attention scores come out as
[ks, qs]; softmax's row-sum falls out of the same matmul that computes ctx via
a shared ones column in v ([v0 | 1 1 | v1]: h0 reads cols 0:65 -> Z at row 64,
h1 reads cols 65:130 -> Z at row 0). rotate_half is a signed-permutation
matmul. v is projected transposed (full-width matmuls) and PE-transposed.

Schedule: chunk-gated waves. Attention kt tiles start as soon as the proj
chunk covering their keys is done, so the exp stream on ACT starts ~9us in.
The kt loop is software-pipelined (ctx lags scores by one kt so PE never
waits on ACT), and proj/out-proj work is interleaved between kt units to
fill PE slack. PSUM: pss double-buffer (2x2 banks, dedicated), psc (2x1,
block-persistent), everything else rotates through 2x1 shared slots.
"""

import os
import sys
from contextlib import ExitStack

import numpy as np

for _p in ("/opt/trn_rl_repo", "/opt/pypackages"):
    if _p not in sys.path and os.path.isdir(_p):
        sys.path.append(_p)

import concourse.bass as bass
import concourse.mybir as mybir
import concourse.tile as tile
from concourse import bacc
from concourse import bass_utils
from concourse.masks import make_identity

F32 = mybir.dt.float32
AF = mybir.ActivationFunctionType
OP = mybir.AluOpType

B = 2
S = 2048
E = 1024
H = 16
D = 64
N_CORES = 8
HPC = H // N_CORES  # heads per core = 2
HD = HPC * D  # 128

MM_MODE = os.environ.get("MHA_MM_MODE", "bf16")

LAST_RESULTS = None  # BassKernelResults of the most recent run (for test harness)
_NC_CACHE = {}

VW = 132  # v tile row width: v0(64) | ones | pad | v1(64) | ones | pad


def build_mha_nc(mm_mode):
    T = B * S
    TC = 512  # token chunk for projections
    NCH = T // TC  # 8
    QC = 512  # query chunk in attention
    NQC = S // QC  # 4
    NKT = S // 128  # 16 key tiles per batch
    KE = E // 128  # 8 contraction tiles for projections

    dt_in = {"bf16": mybir.dt.bfloat16, "f32r": mybir.dt.float32r, "f32": F32}[mm_mode]

    nc = bacc.Bacc(None, target_bir_lowering=False, debug=False)

    xT = nc.dram_tensor("xT", [E, T], dt_in, kind="ExternalInput")
    wq = nc.dram_tensor("wq", [E, HD], dt_in, kind="ExternalInput")
    wk = nc.dram_tensor("wk", [E, HD], dt_in, kind="ExternalInput")
    wv = nc.dram_tensor("wv", [E, HD], dt_in, kind="ExternalInput")
    bq = nc.dram_tensor("bq", [HD, 1], F32, kind="ExternalInput")
    bk = nc.dram_tensor("bk", [HD, 1], F32, kind="ExternalInput")
    bv = nc.dram_tensor("bv", [HD, 1], F32, kind="ExternalInput")
    wo = nc.dram_tensor("wo", [HD, E], dt_in, kind="ExternalInput")
    cosT = nc.dram_tensor("cosT", [D, S], F32, kind="ExternalInput")
    sinT = nc.dram_tensor("sinT", [D, S], F32, kind="ExternalInput")
    rot = nc.dram_tensor("rot", [HD, HD], dt_in, kind="ExternalInput")
    yp = nc.dram_tensor("yp", [T, E], dt_in, kind="ExternalOutput")
    debug = os.environ.get("MHA_DEBUG", "") == "1"
    if debug:
        dbg_q = nc.dram_tensor("dbg_q", [HD, T], dt_in, kind="ExternalOutput")
        dbg_k = nc.dram_tensor("dbg_k", [HD, T], dt_in, kind="ExternalOutput")
        dbg_v = nc.dram_tensor("dbg_v", [T // 128, 128, VW], dt_in, kind="ExternalOutput")
        dbg_cp = nc.dram_tensor("dbg_cp", [B, HD, S], dt_in, kind="ExternalOutput")

    scale = 1.0 / np.sqrt(D)

    with tile.TileContext(nc) as tc, ExitStack() as ctx:
        const = ctx.enter_context(tc.tile_pool(name="const", bufs=1))
        xt_pool = ctx.enter_context(tc.tile_pool(name="xt", bufs=2 * KE))
        cs_pool = ctx.enter_context(tc.tile_pool(name="cs", bufs=4))
        qkraw_pool = ctx.enter_context(tc.tile_pool(name="qkraw", bufs=4))
        rope_tmp = ctx.enter_context(tc.tile_pool(name="ropetmp", bufs=4))
        persist = ctx.enter_context(tc.tile_pool(name="persist", bufs=1))
        exps_pool = ctx.enter_context(tc.tile_pool(name="exps", bufs=8))
        zr_pool = ctx.enter_context(tc.tile_pool(name="zr", bufs=6))
        zb_pool = ctx.enter_context(tc.tile_pool(name="zb", bufs=6))
        osb_pool = ctx.enter_context(tc.tile_pool(name="osb", bufs=6))
        csh_pool = ctx.enter_context(tc.tile_pool(name="csh", bufs=6))
        dram = ctx.enter_context(tc.tile_pool(name="dram", bufs=8, space="DRAM"))

        # PSUM: pss 2x2 banks (dedicated), psc 2x1 (block-persistent),
        # shared 2x1 rotation for psq/psk/psv/psrot/pvt/pso.
        ps_s = ctx.enter_context(tc.tile_pool(name="ps_s", bufs=2, space="PSUM"))
        ps_c = ctx.enter_context(tc.tile_pool(name="ps_c", bufs=2, space="PSUM"))
        ps_w = ctx.enter_context(tc.tile_pool(name="ps_w", bufs=2, space="PSUM"))

        # ---- constants to SBUF (gpsimd queue; off the sync DMA path) ----
        def load_const(name, dram_t, shape, dt):
            t = const.tile(shape, dt, name=name, tag=name)
            nc.gpsimd.dma_start(t[:], dram_t.ap())
            return t

        wq_sb = [None] * KE
        wk_sb = [None] * KE
        wv_sb = [None] * KE
        for k in range(KE):
            for nm, dr, arr in (("wq", wq, wq_sb), ("wk", wk, wk_sb), ("wv", wv, wv_sb)):
                t = const.tile([128, HD], dt_in, name=f"{nm}_{k}", tag=f"{nm}_{k}")
                nc.gpsimd.dma_start(t[:], dr.ap()[128 * k : 128 * (k + 1), :])
                arr[k] = t
        bq_sb = load_const("bq_sb", bq, [HD, 1], F32)
        bk_sb = load_const("bk_sb", bk, [HD, 1], F32)
        bv_sb = load_const("bv_sb", bv, [HD, 1], F32)
        wo_sb = load_const("wo_sb", wo, [HD, E], dt_in)
        rot_sb = load_const("rot_sb", rot, [HD, HD], dt_in)
        ident = const.tile([128, 128], dt_in, name="ident", tag="ident")
        make_identity(nc, ident)

        # ---- persistent intermediates ----
        q_rope = persist.tile([HD, T], dt_in, name="q_rope", tag="q_rope")
        k_rope = persist.tile([HD, T], dt_in, name="k_rope", tag="k_rope")
        # v tiles: [tokens(128), v0(64) | ones(2) | v1(64)] per 128-token tile
        v_sb = []
        for i in range(T // 128):
            t = persist.tile([128, VW], dt_in, name=f"v_{i}", tag=f"v_{i}")
            nc.vector.memset(t[:, 64:65], 1.0)
            nc.vector.memset(t[:, 130:131], 1.0)
            v_sb.append(t)
        ctx_pack = {}
        for b in range(B):
            ctx_pack[b] = persist.tile([HD, S], dt_in, name=f"ctxp_{b}", tag=f"ctxp_{b}")

        # ---- stage 1: projection chunk, split into 3 parts (q, k, v) ----
        def part_qk(c, nm):
            c0 = TC * c
            if nm == "q":  # loads for the whole chunk ride with the q part
                for k in range(KE):
                    t = xt_pool.tile([128, TC], dt_in, name=f"xt_{c}_{k}", tag="xt")
                    nc.sync.dma_start(t[:], xT.ap()[128 * k : 128 * (k + 1), c0 : c0 + TC])
                    xt_cur[c] = xt_cur.get(c, [])
                    xt_cur[c].append(t)
                s0 = c0 % S
                for key, dr in (("cos", cosT), ("sin", sinT)):
                    t = cs_pool.tile([HD, TC], F32, name=f"{key}_c", tag=f"{key}_c")
                    dr_ap = dr.ap()
                    nc.sync.dma_start(
                        t[:],
                        bass.AP(
                            tensor=dr_ap.tensor,
                            offset=dr_ap.offset + s0,
                            ap=[[0, HPC], [S, D], [1, TC]],
                        ),
                    )
                    cs_cur[(c, key)] = t
            xt = xt_cur[c]
            w_sb, b_sb, out = {
                "q": (wq_sb, bq_sb, q_rope),
                "k": (wk_sb, bk_sb, k_rope),
            }[nm]
            ps = ps_w.tile([HD, TC], F32, name=f"ps{nm}", tag="ps_w")
            for k in range(KE):
                nc.tensor.matmul(
                    ps[:], w_sb[k][:], xt[k][:], start=(k == 0), stop=(k == KE - 1)
                )
            raw = qkraw_pool.tile([HD, TC], dt_in, name=f"{nm}raw", tag="qkraw")
            nc.vector.tensor_scalar_add(raw[:], ps[:], b_sb[:, 0:1])
            psrot = ps_w.tile([HD, TC], F32, name="psrot", tag="ps_w")
            nc.tensor.matmul(psrot[:], rot_sb[:], raw[:], start=True, stop=True)
            cprod = rope_tmp.tile([HD, TC], F32, name="cprod", tag="ropetmp")
            nc.vector.tensor_tensor(cprod[:], raw[:], cs_cur[(c, "cos")][:], op=OP.mult)
            sprod = rope_tmp.tile([HD, TC], F32, name="sprod", tag="ropetmp")
            nc.vector.tensor_tensor(sprod[:], psrot[:], cs_cur[(c, "sin")][:], op=OP.mult)
            nc.vector.tensor_tensor(out[:, c0 : c0 + TC], cprod[:], sprod[:], op=OP.add)

        def part_v(c):
            c0 = TC * c
            xt = xt_cur[c]
            psv = ps_w.tile([HD, TC], F32, name="psv", tag="ps_w")
            for k in range(KE):
                nc.tensor.matmul(
                    psv[:], wv_sb[k][:], xt[k][:], start=(k == 0), stop=(k == KE - 1)
                )
            vraw = qkraw_pool.tile([HD, TC], dt_in, name="vraw", tag="qkraw")
            nc.vector.tensor_scalar_add(vraw[:], psv[:], bv_sb[:, 0:1])
            for j in range(TC // 128):
                pvt = ps_w.tile([128, 128], dt_in, name="pvt", tag="ps_w")
                nc.tensor.transpose(pvt[:], vraw[:, 128 * j : 128 * (j + 1)], ident[:])
                vt = v_sb[(c0 + 128 * j) // 128]
                vt_ap = vt[:]
                # one strided copy: psum cols {0:64,64:128} -> vt cols {0:64,66:130}
                dst = bass.AP(
                    tensor=vt_ap.tensor,
                    offset=vt_ap.offset,
                    ap=[vt_ap.ap[0], [66, 2], [1, 64]],
                )
                pvt_ap = pvt[:]
                src = bass.AP(
                    tensor=pvt_ap.tensor,
                    offset=pvt_ap.offset,
                    ap=[pvt_ap.ap[0], [64, 2], [1, 64]],
                )
                nc.vector.tensor_copy(dst, src)

        xt_cur = {}
        cs_cur = {}

        # ---- stage 2: attention, software-pipelined kt loop ----
        def open_block():
            # h0: ctx rows 0:64, Z row 64 ; h1: Z row 0, ctx rows 1:65
            return [
                ps_c.tile([D + 1, QC], F32, name=f"psctx{h}", tag="ps_c")
                for h in range(HPC)
            ]

        def kt_scores(b, qc, kt):
            t0 = b * S
            q0 = t0 + QC * qc
            k0 = t0 + 128 * kt
            pss = ps_s.tile([128, HPC * QC], F32, name="pss", tag="ps_s")
            for h in range(HPC):
                nc.tensor.matmul(
                    pss[:, QC * h : QC * (h + 1)],
                    k_rope[D * h : D * (h + 1), k0 : k0 + 128],
                    q_rope[D * h : D * (h + 1), q0 : q0 + QC],
                    start=True, stop=True,
                )
            ex = exps_pool.tile([128, HPC * QC], dt_in, name="ex", tag="exps")
            nc.scalar.activation(ex[:], pss[:], AF.Exp, scale=scale)
            return ex

        def kt_ctx(b, qc, kt, psc, ex):
            t0 = b * S
            k0 = t0 + 128 * kt
            vt = v_sb[k0 // 128]
            for h in range(HPC):
                nc.tensor.matmul(
                    psc[h][:],
                    vt[:, 66 * h : 66 * h + 65],
                    ex[:, QC * h : QC * (h + 1)],
                    start=(kt == 0), stop=(kt == NKT - 1),
                )

        def finish_block(b, qc, psc):
            # evict unnormalized ctx+Z (f32), 1/Z via fast approx, DRAM-bounce
            # broadcast, normalize-multiply into ctx_pack (bf16)
            for h in range(HPC):
                cun = csh_pool.tile([D + 1, QC], F32, name="cun", tag="csh")
                nc.vector.tensor_copy(cun[:], psc[h][:])
                zrow = zr_pool.tile([1, QC], F32, name="zrow", tag="zrow")
                nc.vector.tensor_copy(zrow[:], cun[D : D + 1, :])
                zr = zr_pool.tile([1, QC], F32, name="zrec", tag="zr")
                nc.vector.reciprocal_approx_fast(zr[:], zrow[:])
                zd = dram.tile([QC], F32, name="zd", tag="zd")
                nc.gpsimd.dma_start(zd[:], zr[:])
                zb = zb_pool.tile([D, QC], F32, name="zb", tag="zb")
                zd_ap = zd[:]
                nc.gpsimd.dma_start(
                    zb[:],
                    bass.AP(
                        tensor=zd_ap.tensor, offset=zd_ap.offset, ap=[[0, D], [1, QC]]
                    ),
                )
                if h == 0:
                    nc.vector.tensor_tensor(
                        ctx_pack[b][0:D, QC * qc : QC * (qc + 1)],
                        cun[0:D, :], zb[:], op=OP.mult,
                    )
                else:
                    csh = csh_pool.tile([D, QC], dt_in, name="csh2", tag="csh2")
                    nc.vector.tensor_tensor(csh[:], cun[0:D, :], zb[:], op=OP.mult)
                    nc.scalar.dma_start(
                        ctx_pack[b][D : 2 * D, QC * qc : QC * (qc + 1)], csh[:]
                    )

        # ---- stage 3: one out-projection tile (j, e) of batch b ----
        def out_tile(b, j, e):
            t0 = b * S
            pso = ps_w.tile([128, 512], F32, name="pso", tag="ps_w")
            nc.tensor.matmul(
                pso[:],
                ctx_pack[b][:, 128 * j : 128 * (j + 1)],
                wo_sb[:, 512 * e : 512 * (e + 1)],
                start=True, stop=True,
            )
            osb = osb_pool.tile([128, 512], dt_in, name="osb", tag="osb")
            nc.vector.tensor_copy(osb[:], pso[:])
            eng = nc.sync if e == 0 else nc.gpsimd
            eng.dma_start(
                yp.ap()[t0 + 128 * j : t0 + 128 * (j + 1), 512 * e : 512 * (e + 1)],
                osb[:],
            )

        # ---- emission: chunk-gated waves with interleaved fillers ----
        def emit_wave(kt_units, fillers):
            """Interleave filler callables evenly among kt unit callables."""
            nf, nk = len(fillers), len(kt_units)
            fi = 0
            for i, ku in enumerate(kt_units):
                ku()
                want = (i + 1) * nf // max(nk, 1)
                while fi < want:
                    fillers[fi]()
                    fi += 1
            while fi < nf:
                fillers[fi]()
                fi += 1

        def block_units(b, qc, psc):
            """Software-pipelined kt units: scores(kt) then ctx(kt-1)."""
            state = {}
            units = []

            def mk_scores(kt):
                def f():
                    state[kt] = kt_scores(b, qc, kt)
                return f

            def mk_ctx(kt):
                def f():
                    kt_ctx(b, qc, kt, psc, state.pop(kt))
                return f

            units.append(mk_scores(0))
            for kt in range(1, NKT):
                units.append(mk_scores(kt))
                units.append(mk_ctx(kt - 1))
            units.append(mk_ctx(NKT - 1))
            return units

        def chunk_parts(c):
            return [
                lambda c=c: part_qk(c, "q"),
                lambda c=c: part_qk(c, "k"),
                lambda c=c: part_v(c),
            ]

        def outproj_parts(b, qc):
            JT = S // 128
            tiles = range(JT * qc // NQC, JT * (qc + 1) // NQC)
            return [
                (lambda b=b, j=j, e=e: out_tile(b, j, e))
                for j in tiles
                for e in range(E // 512)
            ]

        # chunk 0 alone (nothing to overlap yet)
        for p in chunk_parts(0):
            p()
        # block (0,0): kt range gated on chunks; chunk c+1 interleaves with
        # the kt wave that chunk c enabled
        psc00 = open_block()
        u = block_units(0, 0, psc00)
        # units list: index of scores(kt) = 2*kt-? -> split by kt groups of 4:
        # scores(0..3)+ctx(0..2) ~ units[0:7], then 8 units per 4-kt group
        emit_wave(u[0:7], chunk_parts(1))
        emit_wave(u[7:15], chunk_parts(2))
        emit_wave(u[15:23], chunk_parts(3))
        emit_wave(u[23:32], chunk_parts(4))
        finish_block(0, 0, psc00)

        # blocks (0,1..3): full kt runs, interleaved with chunk 5..7 and
        # lagging out-proj of the previous block
        for qc in range(1, NQC):
            psc = open_block()
            fillers = []
            if qc + 4 < NCH:
                fillers += chunk_parts(qc + 4)
            fillers += outproj_parts(0, qc - 1)
            emit_wave(block_units(0, qc, psc), fillers)
            finish_block(0, qc, psc)

        # batch 1 blocks; out-proj keeps lagging one block
        prev = [(0, NQC - 1)]
        for qc in range(NQC):
            psc = open_block()
            pb, pqc = prev[-1]
            emit_wave(block_units(1, qc, psc), outproj_parts(pb, pqc))
            finish_block(1, qc, psc)
            prev.append((1, qc))
        # final out-proj tail
        for p in outproj_parts(1, NQC - 1):
            p()

        if debug:
            nc.sync.dma_start(dbg_q.ap(), q_rope[:])
            nc.sync.dma_start(dbg_k.ap(), k_rope[:])
            for i in range(T // 128):
                nc.sync.dma_start(dbg_v.ap()[i], v_sb[i][:])
            for b in range(B):
                nc.sync.dma_start(dbg_cp.ap()[b], ctx_pack[b][:])

    nc.compile()
    return nc


def _rope_tables():
    inv_freq = 1.0 / (10000.0 ** (np.arange(0, D, 2, dtype=np.float32) / D))
    t = np.arange(S, dtype=np.float32)
    freqs = np.outer(t, inv_freq).astype(np.float32)
    emb = np.concatenate([freqs, freqs], axis=-1)
    return np.cos(emb).astype(np.float32), np.sin(emb).astype(np.float32)


def _rot_matrix():
    R = np.zeros((HD, HD), np.float32)
    for hh in range(HPC):
        for do in range(D):
            po = D * hh + do
            if do < D // 2:
                R[D * hh + do + D // 2, po] = -1.0
            else:
                R[D * hh + do - D // 2, po] = 1.0
    return R


def kernel(x, Wq, bq, Wk, bk, Wv, bv, Wo, bo):
    global LAST_RESULTS
    import ml_dtypes

    x = np.asarray(x, dtype=np.float32)
    Wq, bq = np.asarray(Wq, np.float32), np.asarray(bq, np.float32)
    Wk, bk = np.asarray(Wk, np.float32), np.asarray(bk, np.float32)
    Wv, bv = np.asarray(Wv, np.float32), np.asarray(bv, np.float32)
    Wo, bo = np.asarray(Wo, np.float32), np.asarray(bo, np.float32)

    mode = MM_MODE
    dt_np = ml_dtypes.bfloat16 if mode == "bf16" else np.float32
    T = B * S

    if mode not in _NC_CACHE:
        _NC_CACHE[mode] = build_mha_nc(mode)
    nc = _NC_CACHE[mode]

    xT = np.ascontiguousarray(x.reshape(T, E).T).astype(dt_np)
    cos, sin = _rope_tables()
    cosT = np.ascontiguousarray(cos.T).astype(np.float32)
    sinT = np.ascontiguousarray(sin.T).astype(np.float32)
    R = _rot_matrix().astype(dt_np)

    in_maps = []
    for c in range(N_CORES):
        sl = slice(HD * c, HD * (c + 1))
        in_maps.append(
            {
                "xT": xT,
                "wq": np.ascontiguousarray(Wq[:, sl]).astype(dt_np),
                "wk": np.ascontiguousarray(Wk[:, sl]).astype(dt_np),
                "wv": np.ascontiguousarray(Wv[:, sl]).astype(dt_np),
                "bq": np.ascontiguousarray(bq[sl][:, None]).astype(np.float32),
                "bk": np.ascontiguousarray(bk[sl][:, None]).astype(np.float32),
                "bv": np.ascontiguousarray(bv[sl][:, None]).astype(np.float32),
                "wo": np.ascontiguousarray(Wo[sl, :]).astype(dt_np),
                "cosT": cosT,
                "sinT": sinT,
                "rot": R,
            }
        )

    res = bass_utils.run_bass_kernel_spmd(nc, in_maps, core_ids=list(range(N_CORES)))
    LAST_RESULTS = res

    out = np.zeros((T, E), np.float64)
    for c in range(N_CORES):
        out += res.results[c]["yp"].astype(np.float64)
    out += bo.astype(np.float64)
    return out.astype(np.float32).reshape(B, S, E)


# revision 22
# speedup vs baseline: 1.0487x; 1.0285x over previous
"""Trainium2 Bass kernel for nn_MultiHeadAttention (RoPE MHA, B=2 S=2048 E=1024 H=16).

Sharding: tensor-parallel over heads — 2 heads per core on 8 cores. Each core
computes its heads' q/k/v projections, RoPE, attention, and the partial output
projection (its rows of Wo); the host sums the 8 partials and adds bo.

Device layouts: q/k as [d, token] (transposed) so attention scores come out as
[ks, qs]; softmax's row-sum falls out of the same matmul that computes ctx via
a shared ones column in v ([v0 | 1 1 | v1]: h0 reads cols 0:65 -> Z at row 64,
h1 reads cols 65:130 -> Z at row 0). rotate_half is a signed-permutation
matmul. v is projected transposed (full-width matmuls) and PE-transposed.

Schedule: chunk-gated waves. Attention kt tiles start as soon as the proj
chunk covering their keys is done, so the exp stream on ACT starts ~9us in.
The kt loop is software-pipelined (ctx lags scores by one kt so PE never
waits on ACT), and proj/out-proj work is interleaved between kt units to
fill PE slack. PSUM: pss double-buffer (2x2 banks, dedicated), psc (2x1,
block-persistent), everything else rotates through 2x1 shared slots.
"""

import os
import sys
from contextlib import ExitStack

import numpy as np

for _p in ("/opt/trn_rl_repo", "/opt/pypackages"):
    if _p not in sys.path and os.path.isdir(_p):
        sys.path.append(_p)

import concourse.bass as bass
import concourse.mybir as mybir
import concourse.tile as tile
from concourse import bacc
from concourse import bass_utils
from concourse.masks import make_identity

F32 = mybir.dt.float32
AF = mybir.ActivationFunctionType
OP = mybir.AluOpType

B = 2
S = 2048
E = 1024
H = 16
D = 64
N_CORES = 8
HPC = H // N_CORES  # heads per core = 2
HD = HPC * D  # 128

MM_MODE = os.environ.get("MHA_MM_MODE", "bf16")

LAST_RESULTS = None  # BassKernelResults of the most recent run (for test harness)
_NC_CACHE = {}

VW = 132  # v tile row width: v0(64) | ones | pad | v1(64) | ones | pad


def build_mha_nc(mm_mode):
    T = B * S
    TC = 512  # token chunk for projections
    NCH = T // TC  # 8
    QC = 512  # query chunk in attention
    NQC = S // QC  # 4
    NKT = S // 128  # 16 key tiles per batch
    KE = E // 128  # 8 contraction tiles for projections

    dt_in = {"bf16": mybir.dt.bfloat16, "f32r": mybir.dt.float32r, "f32": F32}[mm_mode]

    nc = bacc.Bacc(None, target_bir_lowering=False, debug=False)

    xT = nc.dram_tensor("xT", [E, T], dt_in, kind="ExternalInput")
    wq = nc.dram_tensor("wq", [E, HD], dt_in, kind="ExternalInput")
    wk = nc.dram_tensor("wk", [E, HD], dt_in, kind="ExternalInput")
    wv = nc.dram_tensor("wv", [E, HD], dt_in, kind="ExternalInput")
    bq = nc.dram_tensor("bq", [HD, 1], F32, kind="ExternalInput")
    bk = nc.dram_tensor("bk", [HD, 1], F32, kind="ExternalInput")
    bv = nc.dram_tensor("bv", [HD, 1], F32, kind="ExternalInput")
    wo = nc.dram_tensor("wo", [HD, E], dt_in, kind="ExternalInput")
    cosT = nc.dram_tensor("cosT", [D, S], F32, kind="ExternalInput")
    sinT = nc.dram_tensor("sinT", [D, S], F32, kind="ExternalInput")
    rot = nc.dram_tensor("rot", [HD, HD], dt_in, kind="ExternalInput")
    yp = nc.dram_tensor("yp", [T, E], dt_in, kind="ExternalOutput")
    debug = os.environ.get("MHA_DEBUG", "") == "1"
    if debug:
        dbg_q = nc.dram_tensor("dbg_q", [HD, T], dt_in, kind="ExternalOutput")
        dbg_k = nc.dram_tensor("dbg_k", [HD, T], dt_in, kind="ExternalOutput")
        dbg_v = nc.dram_tensor("dbg_v", [T // 128, 128, VW], dt_in, kind="ExternalOutput")
        dbg_cp = nc.dram_tensor("dbg_cp", [B, HD, S], dt_in, kind="ExternalOutput")

    scale = 1.0 / np.sqrt(D)

    with tile.TileContext(nc) as tc, ExitStack() as ctx:
        const = ctx.enter_context(tc.tile_pool(name="const", bufs=1))
        xt_pool = ctx.enter_context(tc.tile_pool(name="xt", bufs=2 * KE))
        cs_pool = ctx.enter_context(tc.tile_pool(name="cs", bufs=4))
        qkraw_pool = ctx.enter_context(tc.tile_pool(name="qkraw", bufs=4))
        rope_tmp = ctx.enter_context(tc.tile_pool(name="ropetmp", bufs=4))
        persist = ctx.enter_context(tc.tile_pool(name="persist", bufs=1))
        exps_pool = ctx.enter_context(tc.tile_pool(name="exps", bufs=8))
        zr_pool = ctx.enter_context(tc.tile_pool(name="zr", bufs=6))
        zb_pool = ctx.enter_context(tc.tile_pool(name="zb", bufs=6))
        osb_pool = ctx.enter_context(tc.tile_pool(name="osb", bufs=6))
        csh_pool = ctx.enter_context(tc.tile_pool(name="csh", bufs=6))
        dram = ctx.enter_context(tc.tile_pool(name="dram", bufs=8, space="DRAM"))

        # PSUM: pss 2x2 banks (dedicated), psc 2x1 (block-persistent),
        # shared 2x1 rotation for psq/psk/psv/psrot/pvt/pso.
        ps_s = ctx.enter_context(tc.tile_pool(name="ps_s", bufs=2, space="PSUM"))
        ps_c = ctx.enter_context(tc.tile_pool(name="ps_c", bufs=2, space="PSUM"))
        ps_w = ctx.enter_context(tc.tile_pool(name="ps_w", bufs=2, space="PSUM"))

        # ---- constants to SBUF (gpsimd queue; off the sync DMA path) ----
        def load_const(name, dram_t, shape, dt):
            t = const.tile(shape, dt, name=name, tag=name)
            nc.gpsimd.dma_start(t[:], dram_t.ap())
            return t

        wq_sb = [None] * KE
        wk_sb = [None] * KE
        wv_sb = [None] * KE
        for k in range(KE):
            for nm, dr, arr in (("wq", wq, wq_sb), ("wk", wk, wk_sb), ("wv", wv, wv_sb)):
                t = const.tile([128, HD], dt_in, name=f"{nm}_{k}", tag=f"{nm}_{k}")
                nc.gpsimd.dma_start(t[:], dr.ap()[128 * k : 128 * (k + 1), :])
                arr[k] = t
        bq_sb = load_const("bq_sb", bq, [HD, 1], F32)
        bk_sb = load_const("bk_sb", bk, [HD, 1], F32)
        bv_sb = load_const("bv_sb", bv, [HD, 1], F32)
        wo_sb = load_const("wo_sb", wo, [HD, E], dt_in)
        rot_sb = load_const("rot_sb", rot, [HD, HD], dt_in)
        ident = const.tile([128, 128], dt_in, name="ident", tag="ident")
        make_identity(nc, ident)

        # ---- persistent intermediates ----
        q_rope = persist.tile([HD, T], dt_in, name="q_rope", tag="q_rope")
        k_rope = persist.tile([HD, T], dt_in, name="k_rope", tag="k_rope")
        # v tiles: [tokens(128), v0(64) | ones(2) | v1(64)] per 128-token tile
        v_sb = []
        for i in range(T // 128):
            t = persist.tile([128, VW], dt_in, name=f"v_{i}", tag=f"v_{i}")
            nc.vector.memset(t[:, 64:65], 1.0)
            nc.vector.memset(t[:, 130:131], 1.0)
            v_sb.append(t)
        ctx_pack = {}
        for b in range(B):
            ctx_pack[b] = persist.tile([HD, S], dt_in, name=f"ctxp_{b}", tag=f"ctxp_{b}")
        cs_all = {}
        for key, dr, eng in (("cos", cosT, nc.scalar), ("sin", sinT, nc.gpsimd)):
            t = persist.tile([HD, S], F32, name=f"{key}_all", tag=f"{key}_all")
            dr_ap = dr.ap()
            eng.dma_start(
                t[:],
                bass.AP(
                    tensor=dr_ap.tensor, offset=dr_ap.offset,
                    ap=[[0, HPC], [S, D], [1, S]],
                ),
            )
            cs_all[key] = t

        # ---- stage 1: projection chunk, split into 3 parts (q, k, v) ----
        def part_qk(c, nm):
            c0 = TC * c
            if nm == "q":  # loads for the whole chunk ride with the q part
                eng2 = nc.scalar if c < 4 else nc.gpsimd
                for k in range(KE):
                    t = xt_pool.tile([128, TC], dt_in, name=f"xt_{c}_{k}", tag="xt")
                    eng = nc.sync if k % 2 == 0 else eng2
                    eng.dma_start(t[:], xT.ap()[128 * k : 128 * (k + 1), c0 : c0 + TC])
                    xt_cur[c] = xt_cur.get(c, [])
                    xt_cur[c].append(t)
            xt = xt_cur[c]
            s0 = c0 % S
            w_sb, b_sb, out = {
                "q": (wq_sb, bq_sb, q_rope),
                "k": (wk_sb, bk_sb, k_rope),
            }[nm]
            ps = ps_w.tile([HD, TC], F32, name=f"ps{nm}", tag="ps_w")
            for k in range(KE):
                nc.tensor.matmul(
                    ps[:], w_sb[k][:], xt[k][:], start=(k == 0), stop=(k == KE - 1)
                )
            raw = qkraw_pool.tile([HD, TC], dt_in, name=f"{nm}raw", tag="qkraw")
            nc.vector.tensor_scalar_add(raw[:], ps[:], b_sb[:, 0:1])
            psrot = ps_w.tile([HD, TC], F32, name="psrot", tag="ps_w")
            nc.tensor.matmul(psrot[:], rot_sb[:], raw[:], start=True, stop=True)
            cprod = rope_tmp.tile([HD, TC], F32, name="cprod", tag="ropetmp")
            nc.vector.tensor_tensor(
                cprod[:], raw[:], cs_all["cos"][:, s0 : s0 + TC], op=OP.mult
            )
            sprod = rope_tmp.tile([HD, TC], F32, name="sprod", tag="ropetmp")
            nc.vector.tensor_tensor(
                sprod[:], psrot[:], cs_all["sin"][:, s0 : s0 + TC], op=OP.mult
            )
            nc.vector.tensor_tensor(out[:, c0 : c0 + TC], cprod[:], sprod[:], op=OP.add)

        def part_v(c):
            c0 = TC * c
            xt = xt_cur[c]
            psv = ps_w.tile([HD, TC], F32, name="psv", tag="ps_w")
            for k in range(KE):
                nc.tensor.matmul(
                    psv[:], wv_sb[k][:], xt[k][:], start=(k == 0), stop=(k == KE - 1)
                )
            vraw = qkraw_pool.tile([HD, TC], dt_in, name="vraw", tag="qkraw")
            nc.vector.tensor_scalar_add(vraw[:], psv[:], bv_sb[:, 0:1])
            for j in range(TC // 128):
                pvt = ps_w.tile([128, 128], dt_in, name="pvt", tag="ps_w")
                nc.tensor.transpose(pvt[:], vraw[:, 128 * j : 128 * (j + 1)], ident[:])
                vt = v_sb[(c0 + 128 * j) // 128]
                vt_ap = vt[:]
                # one strided copy: psum cols {0:64,64:128} -> vt cols {0:64,66:130}
                dst = bass.AP(
                    tensor=vt_ap.tensor,
                    offset=vt_ap.offset,
                    ap=[vt_ap.ap[0], [66, 2], [1, 64]],
                )
                pvt_ap = pvt[:]
                src = bass.AP(
                    tensor=pvt_ap.tensor,
                    offset=pvt_ap.offset,
                    ap=[pvt_ap.ap[0], [64, 2], [1, 64]],
                )
                nc.vector.tensor_copy(dst, src)

        xt_cur = {}
        cs_cur = {}

        # ---- stage 2: attention, software-pipelined kt loop ----
        def open_block():
            # h0: ctx rows 0:64, Z row 64 ; h1: Z row 0, ctx rows 1:65
            return [
                ps_c.tile([D + 1, QC], F32, name=f"psctx{h}", tag="ps_c")
                for h in range(HPC)
            ]

        def kt_scores(b, qc, kt):
            t0 = b * S
            q0 = t0 + QC * qc
            k0 = t0 + 128 * kt
            pss = ps_s.tile([128, HPC * QC], F32, name="pss", tag="ps_s")
            for h in range(HPC):
                nc.tensor.matmul(
                    pss[:, QC * h : QC * (h + 1)],
                    k_rope[D * h : D * (h + 1), k0 : k0 + 128],
                    q_rope[D * h : D * (h + 1), q0 : q0 + QC],
                    start=True, stop=True,
                )
            ex = exps_pool.tile([128, HPC * QC], dt_in, name="ex", tag="exps")
            nc.scalar.activation(ex[:], pss[:], AF.Exp, scale=scale)
            return ex

        def kt_ctx(b, qc, kt, psc, ex):
            t0 = b * S
            k0 = t0 + 128 * kt
            vt = v_sb[k0 // 128]
            for h in range(HPC):
                nc.tensor.matmul(
                    psc[h][:],
                    vt[:, 66 * h : 66 * h + 65],
                    ex[:, QC * h : QC * (h + 1)],
                    start=(kt == 0), stop=(kt == NKT - 1),
                )

        def finish_block(b, qc, psc):
            # evict unnormalized ctx+Z (f32), 1/Z via fast approx, DRAM-bounce
            # broadcast, normalize-multiply into ctx_pack (bf16)
            cuns = []
            for h in range(HPC):
                cun = csh_pool.tile([D + 1, QC], F32, name="cun", tag="csh")
                nc.vector.tensor_copy(cun[:], psc[h][:])
                cuns.append(cun)
            for h in range(HPC):
                cun = cuns[h]
                zrow = zr_pool.tile([1, QC], F32, name="zrow", tag="zrow")
                nc.vector.tensor_copy(zrow[:], cun[D : D + 1, :])
                zr = zr_pool.tile([1, QC], F32, name="zrec", tag="zr")
                nc.vector.reciprocal_approx_fast(zr[:], zrow[:])
                zd = dram.tile([QC], F32, name="zd", tag="zd")
                nc.gpsimd.dma_start(zd[:], zr[:])
                zb = zb_pool.tile([D, QC], F32, name="zb", tag="zb")
                zd_ap = zd[:]
                nc.gpsimd.dma_start(
                    zb[:],
                    bass.AP(
                        tensor=zd_ap.tensor, offset=zd_ap.offset, ap=[[0, D], [1, QC]]
                    ),
                )
                if h == 0:
                    nc.vector.tensor_tensor(
                        ctx_pack[b][0:D, QC * qc : QC * (qc + 1)],
                        cun[0:D, :], zb[:], op=OP.mult,
                    )
                else:
                    csh = csh_pool.tile([D, QC], dt_in, name="csh2", tag="csh2")
                    nc.vector.tensor_tensor(csh[:], cun[0:D, :], zb[:], op=OP.mult)
                    nc.scalar.dma_start(
                        ctx_pack[b][D : 2 * D, QC * qc : QC * (qc + 1)], csh[:]
                    )

        # ---- stage 3: one out-projection tile (j, e) of batch b ----
        def out_tile(b, j, e):
            t0 = b * S
            pso = ps_w.tile([128, 512], F32, name="pso", tag="ps_w")
            nc.tensor.matmul(
                pso[:],
                ctx_pack[b][:, 128 * j : 128 * (j + 1)],
                wo_sb[:, 512 * e : 512 * (e + 1)],
                start=True, stop=True,
            )
            osb = osb_pool.tile([128, 512], dt_in, name="osb", tag="osb")
            nc.vector.tensor_copy(osb[:], pso[:])
            eng = nc.sync if e == 0 else nc.gpsimd
            eng.dma_start(
                yp.ap()[t0 + 128 * j : t0 + 128 * (j + 1), 512 * e : 512 * (e + 1)],
                osb[:],
            )

        # ---- emission: chunk-gated waves with interleaved fillers ----
        def emit_wave(kt_units, fillers):
            """Interleave filler callables evenly among kt unit callables."""
            nf, nk = len(fillers), len(kt_units)
            fi = 0
            for i, ku in enumerate(kt_units):
                ku()
                want = (i + 1) * nf // max(nk, 1)
                while fi < want:
                    fillers[fi]()
                    fi += 1
            while fi < nf:
                fillers[fi]()
                fi += 1

        def block_units(b, qc, psc):
            """Software-pipelined kt units: scores(kt) then ctx(kt-1)."""
            state = {}
            units = []

            def mk_scores(kt):
                def f():
                    state[kt] = kt_scores(b, qc, kt)
                return f

            def mk_ctx(kt):
                def f():
                    kt_ctx(b, qc, kt, psc, state.pop(kt))
                return f

            units.append(mk_scores(0))
            for kt in range(1, NKT):
                units.append(mk_scores(kt))
                units.append(mk_ctx(kt - 1))
            units.append(mk_ctx(NKT - 1))
            return units

        def chunk_parts(c):
            return [
                lambda c=c: part_qk(c, "q"),
                lambda c=c: part_qk(c, "k"),
                lambda c=c: part_v(c),
            ]

        def outproj_parts(b, qc):
            JT = S // 128
            tiles = range(JT * qc // NQC, JT * (qc + 1) // NQC)
            return [
                (lambda b=b, j=j, e=e: out_tile(b, j, e))
                for j in tiles
                for e in range(E // 512)
            ]

        # chunk 0 alone (nothing to overlap yet)
        for p in chunk_parts(0):
            p()
        # block (0,0): kt range gated on chunks; chunk c+1 interleaves with
        # the kt wave that chunk c enabled
        psc00 = open_block()
        u = block_units(0, 0, psc00)
        # units list: index of scores(kt) = 2*kt-? -> split by kt groups of 4:
        # scores(0..3)+ctx(0..2) ~ units[0:7], then 8 units per 4-kt group
        emit_wave(u[0:7], chunk_parts(1))
        emit_wave(u[7:15], chunk_parts(2))
        emit_wave(u[15:23], chunk_parts(3))
        emit_wave(u[23:32], chunk_parts(4))
        finish_block(0, 0, psc00)

        # blocks (0,1..3): full kt runs, interleaved with chunk 5..7 and
        # lagging out-proj of the previous block
        for qc in range(1, NQC):
            psc = open_block()
            fillers = []
            if qc + 4 < NCH:
                fillers += chunk_parts(qc + 4)
            fillers += outproj_parts(0, qc - 1)
            emit_wave(block_units(0, qc, psc), fillers)
            finish_block(0, qc, psc)

        # batch 1 blocks; out-proj keeps lagging one block
        prev = [(0, NQC - 1)]
        for qc in range(NQC):
            psc = open_block()
            pb, pqc = prev[-1]
            emit_wave(block_units(1, qc, psc), outproj_parts(pb, pqc))
            finish_block(1, qc, psc)
            prev.append((1, qc))
        # final out-proj tail
        for p in outproj_parts(1, NQC - 1):
            p()

        if debug:
            nc.sync.dma_start(dbg_q.ap(), q_rope[:])
            nc.sync.dma_start(dbg_k.ap(), k_rope[:])
            for i in range(T // 128):
                nc.sync.dma_start(dbg_v.ap()[i], v_sb[i][:])
            for b in range(B):
                nc.sync.dma_start(dbg_cp.ap()[b], ctx_pack[b][:])

    nc.compile()
    return nc


def _rope_tables():
    inv_freq = 1.0 / (10000.0 ** (np.arange(0, D, 2, dtype=np.float32) / D))
    t = np.arange(S, dtype=np.float32)
    freqs = np.outer(t, inv_freq).astype(np.float32)
    emb = np.concatenate([freqs, freqs], axis=-1)
    return np.cos(emb).astype(np.float32), np.sin(emb).astype(np.float32)


def _rot_matrix():
    R = np.zeros((HD, HD), np.float32)
    for hh in range(HPC):
        for do in range(D):
            po = D * hh + do
            if do < D // 2:
                R[D * hh + do + D // 2, po] = -1.0
            else:
                R[D * hh + do - D // 2, po] = 1.0
    return R


def kernel(x, Wq, bq, Wk, bk, Wv, bv, Wo, bo):
    global LAST_RESULTS
    import ml_dtypes

    x = np.asarray(x, dtype=np.float32)
    Wq, bq = np.asarray(Wq, np.float32), np.asarray(bq, np.float32)
    Wk, bk = np.asarray(Wk, np.float32), np.asarray(bk, np.float32)
    Wv, bv = np.asarray(Wv, np.float32), np.asarray(bv, np.float32)
    Wo, bo = np.asarray(Wo, np.float32), np.asarray(bo, np.float32)

    mode = MM_MODE
    dt_np = ml_dtypes.bfloat16 if mode == "bf16" else np.float32
    T = B * S

    if mode not in _NC_CACHE:
        _NC_CACHE[mode] = build_mha_nc(mode)
    nc = _NC_CACHE[mode]

    xT = np.ascontiguousarray(x.reshape(T, E).T).astype(dt_np)
    cos, sin = _rope_tables()
    cosT = np.ascontiguousarray(cos.T).astype(np.float32)
    sinT = np.ascontiguousarray(sin.T).astype(np.float32)
    R = _rot_matrix().astype(dt_np)

    in_maps = []
    for c in range(N_CORES):
        sl = slice(HD * c, HD * (c + 1))
        in_maps.append(
            {
                "xT": xT,
                "wq": np.ascontiguousarray(Wq[:, sl]).astype(dt_np),
                "wk": np.ascontiguousarray(Wk[:, sl]).astype(dt_np),
                "wv": np.ascontiguousarray(Wv[:, sl]).astype(dt_np),
                "bq": np.ascontiguousarray(bq[sl][:, None]).astype(np.float32),
                "bk": np.ascontiguousarray(bk[sl][:, None]).astype(np.float32),
                "bv": np.ascontiguousarray(bv[sl][:, None]).astype(np.float32),
                "wo": np.ascontiguousarray(Wo[sl, :]).astype(dt_np),
                "cosT": cosT,
                "sinT": sinT,
                "rot": R,
            }
        )

    res = bass_utils.run_bass_kernel_spmd(nc, in_maps, core_ids=list(range(N_CORES)))
    LAST_RESULTS = res

    out = np.zeros((T, E), np.float64)
    for c in range(N_CORES):
        out += res.results[c]["yp"].astype(np.float64)
    out += bo.astype(np.float64)
    return out.astype(np.float32).reshape(B, S, E)
